# revision 11
# baseline (speedup 1.0000x reference)
"""Trainium2 Bass kernel for nn_CAM: channel attention (CAM) block.

y = gamma * gelu(conv3x3(attn(x))) + x   with
  q/k/v = 1x1 conv projections (d = C/8 = 32),
  energy[d,e] = sum_n q[d,n] k[e,n]  (n over all H*W positions),
  attn = softmax(max_e(energy) - energy, axis=e),
  out  = attn @ v.

Sharding: 8 cores, 2 per sample (B=4). Each core handles 64 rows of H plus
one halo row. Bottom-half cores receive a vertically flipped tile (and a
dy-flipped conv weight) so the SPMD program is identical on all cores; the
energy partial sums are combined with a pairwise AllReduce (4 KB).
"""
import sys

sys.path.insert(0, "/opt/trn_rl_repo")

import numpy as np
import ml_dtypes

import jax
from jax.sharding import Mesh, PartitionSpec, NamedSharding
from jax.experimental.shard_map import shard_map

import concourse.bacc as bacc
import concourse.tile as tile
from concourse import mybir
import concourse.bass as bass
from concourse.masks import make_identity
from concourse.bass2jax import (
    _bass_exec_p,
    install_neuronx_cc_hook,
    partition_id_tensor,
)

F32 = mybir.dt.float32
F32R = mybir.dt.float32r
BF16 = mybir.dt.bfloat16

C = 256
D = 32
H = 128
W = 128
HE = 65          # rows per core incl. 1 halo row
NE = HE * W      # 8320
NOWN = 64 * W    # 8192 (rows owned by this core)
N_CORES = 8
REPLICA_GROUPS = [[0, 1], [2, 3], [4, 5], [6, 7]]


def make_pools(tc, _ctx):
    return dict(
        consts=_ctx.enter_context(tc.tile_pool(name="consts", bufs=1)),
        big=_ctx.enter_context(tc.tile_pool(name="big", bufs=1)),
        work=_ctx.enter_context(tc.tile_pool(name="work", bufs=3)),
        small=_ctx.enter_context(tc.tile_pool(name="small", bufs=2)),
        ps_mm=_ctx.enter_context(tc.tile_pool(name="ps_mm", bufs=3, space="PSUM")),
        ps_e=_ctx.enter_context(tc.tile_pool(name="ps_e", bufs=1, space="PSUM")),
        ps_c=_ctx.enter_context(tc.tile_pool(name="ps_c", bufs=3, space="PSUM")),
        dram=_ctx.enter_context(tc.tile_pool(name="dram", bufs=1, space="DRAM")),
    )


def build_body(tc, aps, pools, use_cc=True):
    nc = tc.nc
    xe, wqkT, wvT, bqk8, bvv, wpp, gam, y = (
        aps["xe"], aps["wqkT"], aps["wvT"], aps["bqk8"], aps["bv"],
        aps["wpp"], aps["gamma"], aps["y"],
    )
    xe_f = xe.rearrange("c h w -> c (h w)")          # [256, 8320]
    y_f = y.rearrange("c h w -> c (h w)")            # [256, 8192]

    consts, big, work, small = (pools["consts"], pools["big"], pools["work"],
                                pools["small"])
    ps_mm, ps_e, ps_c, dram = (pools["ps_mm"], pools["ps_e"], pools["ps_c"],
                               pools["dram"])

    # ---- weights / constants to SBUF ----
    wqk_sb = consts.tile([128, 2, 64], F32)
    wv_sb = consts.tile([128, 2, 32], BF16)
    for c in range(2):
        nc.sync.dma_start(out=wqk_sb[:, c, :], in_=wqkT[c])
        nc.sync.dma_start(out=wv_sb[:, c, :], in_=wvT[c])
    bqk_sb = consts.tile([128, 512], F32)
    nc.sync.dma_start(
        out=bqk_sb[:],
        in_=bass.AP(tensor=bqk8.tensor, offset=bqk8.offset, ap=[[0, 128], [1, 512]]),
    )
    bv_sb = consts.tile([32, 1], F32)
    nc.sync.dma_start(
        out=bv_sb[:],
        in_=bass.AP(tensor=bvv.tensor, offset=bvv.offset, ap=[[1, 32], [1, 1]]),
    )
    gam_sb = consts.tile([128, 1], F32)
    nc.sync.dma_start(
        out=gam_sb[:],
        in_=bass.AP(tensor=gam.tensor, offset=gam.offset, ap=[[0, 128], [1, 1]]),
    )
    wpp_sb = consts.tile([96, 3, 256], BF16)
    for dy in range(3):
        nc.sync.dma_start(out=wpp_sb[:, dy, :], in_=wpp[dy])
    ident = consts.tile([32, 32], F32)
    make_identity(nc, ident)

    # ---- x to SBUF (two 128-channel halves), chunked for pipelining ----
    x0 = big.tile([128, NE], F32)
    x1 = big.tile([128, NE], F32)
    xb0 = big.tile([128, NE], BF16)
    xb1 = big.tile([128, NE], BF16)
    NCHUNK = 8
    csz = NE // NCHUNK  # 1040
    for j in range(NCHUNK):
        s = j * csz
        nc.sync.dma_start(out=x0[:, s:s + csz], in_=xe_f[0:128, s:s + csz])
        nc.sync.dma_start(out=x1[:, s:s + csz], in_=xe_f[128:256, s:s + csz])
        nc.gpsimd.tensor_copy(out=xb0[:, s:s + csz], in_=x0[:, s:s + csz])
        nc.gpsimd.tensor_copy(out=xb1[:, s:s + csz], in_=x1[:, s:s + csz])

    v_sb = big.tile([32, NE], BF16)
    qkt_sb = big.tile([128, 64, 64], F32)
    pa3 = big.tile([96, 66, 130], BF16)

    # ---- V projection (bf16) over all 65 rows ----
    nv = (NE + 511) // 512  # 17
    for i in range(nv):
        s = i * 512
        w = min(512, NE - s)
        vp = ps_mm.tile([32, 512], F32, tag="mm")
        nc.tensor.matmul(vp[:, :w], wv_sb[:, 0, :], xb0[:, s:s + w],
                         start=True, stop=False)
        nc.tensor.matmul(vp[:, :w], wv_sb[:, 1, :], xb1[:, s:s + w],
                         start=False, stop=True)
        nc.vector.tensor_scalar(out=v_sb[:, s:s + w], in0=vp[:, :w],
                                scalar1=bv_sb[:], scalar2=None,
                                op0=mybir.AluOpType.add)

    # ---- Q^T/K^T direct (fp32; x block is the stationary operand) ----
    # qkt_sb[:, b, 0:32] = Q^T rows b*128..b*128+127; [:, b, 32:64] = K^T
    for g in range(8):
        qp = ps_mm.tile([128, 512], F32, tag="mm")
        for j in range(8):
            b = g * 8 + j
            s = b * 128
            nc.tensor.matmul(qp[:, j * 64:(j + 1) * 64], x0[:, s:s + 128],
                             wqk_sb[:, 0, :], start=True, stop=False)
            nc.tensor.matmul(qp[:, j * 64:(j + 1) * 64], x1[:, s:s + 128],
                             wqk_sb[:, 1, :], start=False, stop=True)
        nc.vector.tensor_tensor(
            out=qkt_sb[:, g * 8:(g + 1) * 8, :].rearrange("p a b -> p (a b)"),
            in0=qp[:], in1=bqk_sb[:], op=mybir.AluOpType.add)

    # ---- energy = sum_b QT_b^T @ KT_b  (fp32, accumulate in PSUM) ----
    ep = ps_e.tile([32, 32], F32, tag="e")
    for b in range(64):
        nc.tensor.matmul(ep[:], qkt_sb[:, b, 0:32], qkt_sb[:, b, 32:64],
                         start=(b == 0), stop=(b == 63))
    e_sb = small.tile([32, 32], F32, tag="esb")
    nc.vector.tensor_copy(out=e_sb[:], in_=ep[:])

    # ---- AllReduce energy across the sample pair ----
    E_sb = small.tile([32, 32], F32, tag="Esb")
    if use_cc:
        ein = dram.tile([32, 32], F32)
        eout = dram.tile([32, 32], F32)
        nc.gpsimd.dma_start(out=ein[:], in_=e_sb[:])
        nc.gpsimd.collective_compute(
            "AllReduce", mybir.AluOpType.add, replica_groups=REPLICA_GROUPS,
            ins=[ein.opt()], outs=[eout.opt()])
        nc.gpsimd.dma_start(out=E_sb[:], in_=eout[:])
    else:
        # timing-only variant: collectives inside For_i desync the mesh
        nc.gpsimd.tensor_copy(out=E_sb[:], in_=e_sb[:])

    # ---- softmax over e of (max - E) == softmax(-E), stable via min ----
    rmin = small.tile([32, 1], F32, tag="rmin")
    nc.vector.tensor_reduce(out=rmin[:], in_=E_sb[:], axis=mybir.AxisListType.X,
                            op=mybir.AluOpType.min)
    t_sb = small.tile([32, 32], F32, tag="tsb")
    nc.vector.tensor_scalar(out=t_sb[:], in0=E_sb[:], scalar1=rmin[:],
                            scalar2=None, op0=mybir.AluOpType.subtract)
    p_sb = small.tile([32, 32], F32, tag="psb")
    nc.scalar.activation(out=p_sb[:], in_=t_sb[:],
                         func=mybir.ActivationFunctionType.Exp, scale=-1.0)
    ssum = small.tile([32, 1], F32, tag="ssum")
    nc.vector.reduce_sum(out=ssum[:], in_=p_sb[:], axis=mybir.AxisListType.X)
    rs = small.tile([32, 1], F32, tag="rs")
    nc.vector.reciprocal(out=rs[:], in_=ssum[:])
    attn_sb = small.tile([32, 32], F32, tag="attn")
    nc.vector.tensor_scalar(out=attn_sb[:], in0=p_sb[:], scalar1=rs[:],
                            scalar2=None, op0=mybir.AluOpType.mult)
    atp = ps_e.tile([32, 32], F32, tag="e")
    nc.tensor.transpose(atp[:], attn_sb[:], ident[:])
    attnT = small.tile([32, 32], BF16, tag="attnT")
    nc.vector.tensor_copy(out=attnT[:], in_=atp[:])

    # ---- attnout = attnT.T @ V -> PA3 middle block; DMA-replicate shifts --
    nc.vector.memset(pa3[:, 0, :], 0.0)          # top zero row (h=0)
    nc.vector.memset(pa3[0:32, :, 1], 0.0)       # left pad col for dx=0 block
    nc.vector.memset(pa3[64:96, :, 128], 0.0)    # right pad col for dx=2 block
    for i in range(nv):
        s = i * 512
        w = min(512, NE - s)
        nh = w // 128
        r0 = s // 128
        ap_ = ps_mm.tile([32, 512], F32, tag="mm")
        nc.tensor.matmul(ap_[:, :w], attnT[:], v_sb[:, s:s + w],
                         start=True, stop=True)
        nc.vector.tensor_copy(
            out=pa3[32:64, 1 + r0:1 + r0 + nh, 1:129],
            in_=ap_[:, :w].rearrange("p (h w) -> p h w", w=128))
        nc.scalar.dma_start(out=pa3[0:32, 1 + r0:1 + r0 + nh, 2:130],
                            in_=pa3[32:64, 1 + r0:1 + r0 + nh, 1:129])
        nc.scalar.dma_start(out=pa3[64:96, 1 + r0:1 + r0 + nh, 0:128],
                            in_=pa3[32:64, 1 + r0:1 + r0 + nh, 1:129])

    # ---- conv 3x3 (bf16) + exact gelu + gamma*out + x, then store ----
    for t in range(16):
        for half in range(2):
            xh = x0 if half == 0 else x1
            cp = ps_c.tile([128, 512], F32)
            for dy in range(3):
                nc.tensor.matmul(cp[:], wpp_sb[:, dy, half * 128:(half + 1) * 128],
                                 pa3[:, 4 * t + dy:4 * t + dy + 4, 1:129],
                                 start=(dy == 0), stop=(dy == 2))
            yt = work.tile([128, 512], F32, tag="yt")
            nc.scalar.activation(out=yt[:], in_=cp[:],
                                 func=mybir.ActivationFunctionType.Gelu)
            yo = work.tile([128, 512], F32, tag="yo")
            nc.vector.scalar_tensor_tensor(
                out=yo[:], in0=yt[:], scalar=gam_sb[:],
                in1=xh[:, 512 * t:512 * t + 512],
                op0=mybir.AluOpType.mult, op1=mybir.AluOpType.add)
            nc.sync.dma_start(out=y_f[half * 128:(half + 1) * 128,
                                      512 * t:512 * t + 512], in_=yo[:])


def build_nc(loop_k=None, use_cc=True):
    nc = bacc.Bacc("TRN2", target_bir_lowering=False, debug=False,
                   num_devices=N_CORES)
    aps = {
        "xe": nc.dram_tensor("xe", [C, HE, W], F32, kind="ExternalInput").ap(),
        "wqkT": nc.dram_tensor("wqkT", [2, 128, 64], F32, kind="ExternalInput").ap(),
        "wvT": nc.dram_tensor("wvT", [2, 128, 32], BF16, kind="ExternalInput").ap(),
        "bqk8": nc.dram_tensor("bqk8", [512], F32, kind="ExternalInput").ap(),
        "bv": nc.dram_tensor("bv", [D], F32, kind="ExternalInput").ap(),
        "wpp": nc.dram_tensor("wpp", [3, 96, C], BF16, kind="ExternalInput").ap(),
        "gamma": nc.dram_tensor("gamma", [1], F32, kind="ExternalInput").ap(),
        "y": nc.dram_tensor("y", [C, 64, W], F32, kind="ExternalOutput").ap(),
    }
    with tile.TileContext(nc) as tc:
        from contextlib import ExitStack
        with ExitStack() as _ctx:
            pools = make_pools(tc, _ctx)
            if loop_k is None:
                build_body(tc, aps, pools, use_cc)
            else:
                with tc.For_i(0, loop_k, 1):
                    build_body(tc, aps, pools, use_cc)
    nc.finalize()
    return nc


class SpmdRunner:
    def __init__(self, nc, n_cores):
        install_neuronx_cc_hook()
        self.nc = nc
        self.n_cores = n_cores
        partition_name = nc.partition_id_tensor.name if nc.partition_id_tensor else None
        in_names, out_names, out_avals, zero_outs = [], [], [], []
        for alloc in nc.m.functions[0].allocations:
            if not isinstance(alloc, mybir.MemoryLocationSet):
                continue
            name = alloc.memorylocations[0].name
            if alloc.kind == "ExternalInput":
                if name != partition_name:
                    in_names.append(name)
            elif alloc.kind == "ExternalOutput":
                shape = tuple(alloc.tensor_shape)
                dtype = mybir.dt.np(alloc.dtype)
                out_names.append(name)
                out_avals.append(jax.core.ShapedArray(shape, dtype))
                zero_outs.append(np.zeros(shape, dtype))
        self.in_names, self.out_names = in_names, out_names
        self.out_avals, self.zero_outs = out_avals, zero_outs
        self.n_params = len(in_names)
        all_in = list(in_names) + list(out_names)
        if partition_name is not None:
            all_in.append(partition_name)

        def _body(*args):
            operands = list(args)
            if partition_name is not None:
                operands.append(partition_id_tensor())
            return tuple(_bass_exec_p.bind(
                *operands, out_avals=tuple(out_avals), in_names=tuple(all_in),
                out_names=tuple(out_names), lowering_input_output_aliases=(),
                sim_require_finite=False, sim_require_nnan=False, nc=nc))

        devices = jax.devices()[:n_cores]
        self.mesh = Mesh(np.asarray(devices), ("core",))
        n_outs = len(out_avals)
        in_specs = (PartitionSpec("core"),) * (self.n_params + n_outs)
        out_specs = (PartitionSpec("core"),) * n_outs
        self.sharded = jax.jit(
            shard_map(_body, mesh=self.mesh, in_specs=in_specs,
                      out_specs=out_specs, check_rep=False),
            keep_unused=True)

    def prepare(self, in_maps):
        n = self.n_cores
        concat_in = [
            np.concatenate([np.asarray(in_maps[c][k]) for c in range(n)], axis=0)
            for k in self.in_names
        ]
        concat_zero = [np.zeros((n * z.shape[0], *z.shape[1:]), z.dtype)
                       for z in self.zero_outs]
        sh = NamedSharding(self.mesh, PartitionSpec("core"))
        return [jax.device_put(a, sh) for a in concat_in + concat_zero]

    def run(self, args):
        outs = self.sharded(*args)
        jax.block_until_ready(outs)
        return outs

    def results(self, outs):
        n = self.n_cores
        return [
            {name: np.asarray(outs[i]).reshape(n, *self.out_avals[i].shape)[c]
             for i, name in enumerate(self.out_names)}
            for c in range(n)
        ]


_RUNNER_CACHE = {}


def get_runner(loop_k=None, use_cc=True):
    key = (loop_k, use_cc)
    if key not in _RUNNER_CACHE:
        _RUNNER_CACHE[key] = SpmdRunner(build_nc(loop_k, use_cc), N_CORES)
    return _RUNNER_CACHE[key]


def make_in_maps(x, wq, bq, wk, bk, wv, bv, wp, gamma):
    """Shard FULL inputs into 8 per-core input dicts (with flip trick)."""
    B = x.shape[0]
    wqkT = np.ascontiguousarray(
        np.concatenate([wq.T, wk.T], axis=1).reshape(2, 128, 64), np.float32)
    wvT = np.ascontiguousarray(wv.T.reshape(2, 128, 32)).astype(ml_dtypes.bfloat16)
    bqk8 = np.tile(np.concatenate([bq, bk]), 8).astype(np.float32)
    wpp_n = np.ascontiguousarray(
        np.transpose(wp, (2, 3, 1, 0)).reshape(3, 96, 256)).astype(ml_dtypes.bfloat16)
    wp_fl = wp[:, :, ::-1, :]
    wpp_f = np.ascontiguousarray(
        np.transpose(wp_fl, (2, 3, 1, 0)).reshape(3, 96, 256)).astype(ml_dtypes.bfloat16)
    gam = gamma.astype(np.float32)
    bvf = bv.astype(np.float32)

    in_maps = []
    for b in range(B):
        top = np.ascontiguousarray(x[b, :, 0:HE, :], np.float32)
        bot = np.ascontiguousarray(x[b, :, W - 1:W - 1 - HE:-1, :], np.float32)
        for xe, wpp in ((top, wpp_n), (bot, wpp_f)):
            in_maps.append(dict(xe=xe, wqkT=wqkT, wvT=wvT, bqk8=bqk8, bv=bvf,
                                wpp=wpp, gamma=gam))
    return in_maps


def assemble(results):
    """Gather per-core [256, 64, 128] outputs into [4, 256, 128, 128]."""
    B = len(results) // 2
    y = np.empty((B, C, H, W), np.float32)
    for b in range(B):
        y[b, :, 0:64, :] = results[2 * b]["y"]
        y[b, :, 64:128, :] = results[2 * b + 1]["y"][:, ::-1, :]
    return y


def kernel(**inputs):
    r = get_runner(None)
    in_maps = make_in_maps(**inputs)
    args = r.prepare(in_maps)
    outs = r.run(args)
    return assemble(r.results(outs))


# revision 16
# speedup vs baseline: 2.7766x; 2.7766x over previous
"""Trainium2 Bass kernel for nn_CAM: channel attention (CAM) block.

y = gamma * gelu(conv3x3(attn(x))) + x   with
  q/k/v = 1x1 conv projections (d = C/8 = 32),
  energy[d,e] = sum_n q[d,n] k[e,n]  (n over all H*W positions),
  attn = softmax(max_e(energy) - energy, axis=e)  (== softmax(-energy)),
  out  = attn @ v.

Sharding: 8 cores, 2 per sample (B=4). Each core handles 64 rows of H plus
one halo row. Bottom-half cores receive a vertically flipped tile (and a
dy-flipped conv weight) so the SPMD program is identical on all cores; the
energy partial sums are combined with a pairwise AllReduce (4 KB).

Kernel pipeline (per core):
  x (fp32, DMA) -> staging -> rounded float32r copy (gpsimd)   [only x_r kept]
  QK = wqk_r.T @ x_r          float32r matmuls, N=512 tiles (1 cyc/row)
  QK + bias -> bf16 hi/lo split (DVE)  -> DMA-transpose -> [n,64] chunks
  energy = sum_b (Qh^T Kh + Qh^T Kl + Ql^T Kh)   bf16 matmuls, exact split
  energy -> pairwise AllReduce -> softmax(-E) -> attn^T (PE transpose)
  V = wv_r.T @ x_r (float32r) -> +bv -> bf16     (emitted in the CC window)
  attnout = attn^T.T @ V -> padded conv buffer (middle dx block)
            -> two shifted SBUF->SBUF DMA copies (dx=0,2 partition blocks)
  conv3x3 = 3 accumulating K=96 bf16 matmuls per [128,512] tile
  y = gamma * gelu(conv) + x_r    (ACT gelu + DVE fused mul-add), DMA out
"""
import sys

sys.path.insert(0, "/opt/trn_rl_repo")

from contextlib import ExitStack

import numpy as np
import ml_dtypes

import jax
from jax.sharding import Mesh, PartitionSpec, NamedSharding
from jax.experimental.shard_map import shard_map

import concourse.bacc as bacc
import concourse.tile as tile
from concourse import mybir
import concourse.bass as bass
from concourse.masks import make_identity
from concourse.bass2jax import (
    _bass_exec_p,
    install_neuronx_cc_hook,
    partition_id_tensor,
)

F32 = mybir.dt.float32
F32R = mybir.dt.float32r
BF16 = mybir.dt.bfloat16
OP = mybir.AluOpType
AF = mybir.ActivationFunctionType

C = 256
D = 32
H = 128
W = 128
HE = 65          # rows per core incl. 1 halo row
NE = HE * W      # 8320
NOWN = 64 * W    # 8192 (rows owned by this core)
NB = 64          # 128-col blocks over own rows
N_CORES = 8
REPLICA_GROUPS = [[0, 1], [2, 3], [4, 5], [6, 7]]


def make_pools(tc, _ctx):
    return dict(
        consts=_ctx.enter_context(tc.tile_pool(name="consts", bufs=1)),
        big=_ctx.enter_context(tc.tile_pool(name="big", bufs=1)),
        stage=_ctx.enter_context(tc.tile_pool(name="stage", bufs=4)),
        work=_ctx.enter_context(tc.tile_pool(name="work", bufs=3)),
        small=_ctx.enter_context(tc.tile_pool(name="small", bufs=2)),
        ps_mm=_ctx.enter_context(tc.tile_pool(name="ps_mm", bufs=2, space="PSUM")),
        ps_qk=_ctx.enter_context(tc.tile_pool(name="ps_qk", bufs=4, space="PSUM")),
        ps_e=_ctx.enter_context(tc.tile_pool(name="ps_e", bufs=1, space="PSUM")),
        dram=_ctx.enter_context(tc.tile_pool(name="dram", bufs=1, space="DRAM")),
    )


def _store_passthrough(nc, y_f, xr0, xr1):
    for t in range(16):
        for half, xh in ((0, xr0), (1, xr1)):
            nc.sync.dma_start(
                out=y_f[half * 128:(half + 1) * 128, 512 * t:512 * t + 512],
                in_=xh[:, 512 * t:512 * t + 512].bitcast(F32))


def build_body(tc, aps, pools, use_cc=True, parts=None):
    parts = parts or {"v", "qkt", "attn", "conv"}
    nc = tc.nc
    xe, wqkT, wvT, bqk, bvv, wpp, gam, y = (
        aps["xe"], aps["wqkT"], aps["wvT"], aps["bqk"], aps["bv"],
        aps["wpp"], aps["gamma"], aps["y"],
    )
    xe_f = xe.rearrange("c h w -> c (h w)")          # [256, 8320]
    y_f = y.rearrange("c h w -> c (h w)")            # [256, 8192]

    consts, big, stage, work, small = (pools["consts"], pools["big"],
                                       pools["stage"], pools["work"],
                                       pools["small"])
    ps_mm, ps_qk, ps_e, dram = (pools["ps_mm"], pools["ps_qk"], pools["ps_e"],
                                pools["dram"])

    # ---- weights / constants ----
    wqk_st = consts.tile([128, 2, 64], F32, tag="wqkst")
    wv_st = consts.tile([128, 2, 32], F32, tag="wvst")
    for c in range(2):
        nc.sync.dma_start(out=wqk_st[:, c, :], in_=wqkT[c])
        nc.sync.dma_start(out=wv_st[:, c, :], in_=wvT[c])
    wqk_r = consts.tile([128, 2, 64], F32R, tag="wqkr")
    wv_r = consts.tile([128, 2, 32], F32R, tag="wvr")
    nc.gpsimd.tensor_copy(out=wqk_r[:], in_=wqk_st[:])
    nc.gpsimd.tensor_copy(out=wv_r[:], in_=wv_st[:])
    bqk_sb = consts.tile([64, 1], F32)
    nc.sync.dma_start(
        out=bqk_sb[:],
        in_=bass.AP(tensor=bqk.tensor, offset=bqk.offset, ap=[[1, 64], [1, 1]]))
    bv_sb = consts.tile([32, 1], F32)
    nc.sync.dma_start(
        out=bv_sb[:],
        in_=bass.AP(tensor=bvv.tensor, offset=bvv.offset, ap=[[1, 32], [1, 1]]))
    gam_sb = consts.tile([128, 1], F32)
    nc.sync.dma_start(
        out=gam_sb[:],
        in_=bass.AP(tensor=gam.tensor, offset=gam.offset, ap=[[0, 128], [1, 1]]))
    wpp_sb = consts.tile([96, 3, 256], BF16)
    for dy in range(3):
        nc.sync.dma_start(out=wpp_sb[:, dy, :], in_=wpp[dy])
    ident = consts.tile([32, 32], F32)
    make_identity(nc, ident)

    # ---- x load + round to float32r (only the rounded copy is kept) ----
    xr0 = big.tile([128, NE], F32R)
    xr1 = big.tile([128, NE], F32R)
    NCHUNK = 8
    csz = NE // NCHUNK  # 1040
    for j in range(NCHUNK):
        s = j * csz
        for xrh, lo in ((xr0, 0), (xr1, 128)):
            st = stage.tile([128, csz], F32, tag="xstage")
            nc.sync.dma_start(out=st[:], in_=xe_f[lo:lo + 128, s:s + csz])
            nc.gpsimd.tensor_copy(out=xrh[:, s:s + csz], in_=st[:])

    v_sb = big.tile([32, NE], BF16)
    qk2 = big.tile([64, 2, NOWN], BF16, tag="bigshare")     # [ Q|K , h|l , n ]
    qkt = big.tile([128, 128, 64], BF16)                    # transposed chunks
    nv = (NE + 511) // 512  # 17

    # ---- QK = wqk_r.T @ x_r  (float32r, weight-stationary, N=512) ----
    if "qkt" in parts:
        for i in range(16):
            sl = slice(i * 512, (i + 1) * 512)
            qp = ps_qk.tile([64, 512], F32, tag="qk")
            nc.tensor.matmul(qp[:], wqk_r[:, 0, :], xr0[:, sl],
                             start=True, stop=False)
            nc.tensor.matmul(qp[:], wqk_r[:, 1, :], xr1[:, sl],
                             start=False, stop=True)
            # hi/lo bf16 split with bias folded in
            nc.vector.tensor_scalar(out=qk2[:, 0, sl], in0=qp[:],
                                    scalar1=bqk_sb[:], scalar2=None, op0=OP.add)
            nc.vector.scalar_tensor_tensor(out=qk2[:, 1, sl], in0=qp[:],
                                           scalar=bqk_sb[:], in1=qk2[:, 0, sl],
                                           op0=OP.add, op1=OP.subtract)

        # ---- transpose: 4 chunks x 2 splits -> qkt[:, s*64 + b, :] ----
        TCH = 4
        tsz = NOWN // TCH  # 2048 -> 16 blocks per call
        for s in range(2):
            for j in range(TCH):
                sl = slice(j * tsz, (j + 1) * tsz)
                nc.scalar.dma_start_transpose(
                    qkt[:, s * 64 + j * 16:s * 64 + (j + 1) * 16, :],
                    qk2[:, s, sl])

        # ---- energy: 3 exact split terms, two PSUM accumulation groups ----
        e1 = ps_e.tile([32, 64], F32, tag="e1")
        e2 = ps_e.tile([32, 32], F32, tag="e2")
        part = qkt[:].ap[0][0]
        for b in range(NB):
            rhs2 = bass.AP(tensor=qkt.tensor, offset=qkt[:, b, 32:64].offset,
                           ap=[[part, 128], [64 * 64, 2], [1, 32]])
            nc.tensor.matmul(e1[:], qkt[:, b, 0:32], rhs2,
                             start=(b == 0), stop=(b == NB - 1))
            nc.tensor.matmul(e2[:], qkt[:, 64 + b, 0:32], qkt[:, b, 32:64],
                             start=(b == 0), stop=(b == NB - 1))
        e1s = small.tile([32, 64], F32, tag="e1s")
        nc.vector.tensor_copy(out=e1s[:], in_=e1[:])
        e12 = small.tile([32, 32], F32, tag="e12")
        nc.vector.tensor_tensor(out=e12[:], in0=e1s[:, 0:32], in1=e1s[:, 32:64],
                                op=OP.add)
        e_sb = small.tile([32, 32], F32, tag="esb")
        nc.vector.tensor_tensor(out=e_sb[:], in0=e12[:], in1=e2[:], op=OP.add)

        # ---- AllReduce energy across the sample pair ----
        E_sb = small.tile([32, 32], F32, tag="Esb")
        if use_cc:
            ein = dram.tile([32, 32], F32)
            eout = dram.tile([32, 32], F32)
            nc.gpsimd.dma_start(out=ein[:], in_=e_sb[:])
            nc.gpsimd.collective_compute(
                "AllReduce", OP.add, replica_groups=REPLICA_GROUPS,
                ins=[ein.opt()], outs=[eout.opt()])
            nc.gpsimd.dma_start(out=E_sb[:], in_=eout[:])
        else:
            nc.gpsimd.tensor_copy(out=E_sb[:], in_=e_sb[:])

    # ---- V projection (float32r) over all 65 rows; fills the CC window ----
    if "v" in parts:
        for i in range(nv):
            s = i * 512
            w = min(512, NE - s)
            vp = ps_mm.tile([32, 512], F32, tag="mm")
            nc.tensor.matmul(vp[:, :w], wv_r[:, 0, :], xr0[:, s:s + w],
                             start=True, stop=False)
            nc.tensor.matmul(vp[:, :w], wv_r[:, 1, :], xr1[:, s:s + w],
                             start=False, stop=True)
            nc.scalar.activation(out=v_sb[:, s:s + w], in_=vp[:, :w],
                                 func=AF.Identity, bias=bv_sb[:], scale=1.0)

    if "qkt" not in parts or "attn" not in parts:
        return _store_passthrough(nc, y_f, xr0, xr1)

    # ---- softmax over e of -E, stable via min ----
    rmin = small.tile([32, 1], F32, tag="rmin")
    nc.vector.tensor_reduce(out=rmin[:], in_=E_sb[:], axis=mybir.AxisListType.X,
                            op=OP.min)
    t_sb = small.tile([32, 32], F32, tag="tsb")
    nc.vector.tensor_scalar(out=t_sb[:], in0=E_sb[:], scalar1=rmin[:],
                            scalar2=None, op0=OP.subtract)
    p_sb = small.tile([32, 32], F32, tag="psb")
    nc.scalar.activation(out=p_sb[:], in_=t_sb[:], func=AF.Exp, scale=-1.0)
    ssum = small.tile([32, 1], F32, tag="ssum")
    nc.vector.reduce_sum(out=ssum[:], in_=p_sb[:], axis=mybir.AxisListType.X)
    rs = small.tile([32, 1], F32, tag="rs")
    nc.vector.reciprocal(out=rs[:], in_=ssum[:])
    attn_sb = small.tile([32, 32], F32, tag="attn")
    nc.vector.tensor_scalar(out=attn_sb[:], in0=p_sb[:], scalar1=rs[:],
                            scalar2=None, op0=OP.mult)
    atp = ps_e.tile([32, 32], F32, tag="e2")
    nc.tensor.transpose(atp[:], attn_sb[:], ident[:])
    attnT = small.tile([32, 32], BF16, tag="attnT")
    nc.vector.tensor_copy(out=attnT[:], in_=atp[:])

    # ---- attnout -> PA3 middle block; DMA-replicate w-shifted copies ----
    pa3 = big.tile([96, 66, 130], BF16, tag="bigshare")
    nc.vector.memset(pa3[:, 0, :], 0.0)          # top zero row (h=0)
    nc.vector.memset(pa3[0:32, :, 1], 0.0)       # left pad col (dx=0 block)
    nc.vector.memset(pa3[64:96, :, 128], 0.0)    # right pad col (dx=2 block)
    for i in range(nv):
        s = i * 512
        w = min(512, NE - s)
        nh = w // 128
        r0 = s // 128
        ap_ = ps_mm.tile([32, 512], F32, tag="mm")
        nc.tensor.matmul(ap_[:, :w], attnT[:], v_sb[:, s:s + w],
                         start=True, stop=True)
        nc.vector.tensor_copy(
            out=pa3[32:64, 1 + r0:1 + r0 + nh, 1:129],
            in_=ap_[:, :w].rearrange("p (h w) -> p h w", w=128))
        nc.scalar.dma_start(out=pa3[0:32, 1 + r0:1 + r0 + nh, 2:130],
                            in_=pa3[32:64, 1 + r0:1 + r0 + nh, 1:129])
        nc.scalar.dma_start(out=pa3[64:96, 1 + r0:1 + r0 + nh, 0:128],
                            in_=pa3[32:64, 1 + r0:1 + r0 + nh, 1:129])

    if "conv" not in parts:
        return _store_passthrough(nc, y_f, xr0, xr1)

    # ---- conv 3x3 (bf16) + exact gelu + gamma*out + x, then store ----
    for t in range(16):
        for half in range(2):
            xh = xr0 if half == 0 else xr1
            cp = ps_mm.tile([128, 512], F32, tag="mm")
            for dy in range(3):
                nc.tensor.matmul(cp[:], wpp_sb[:, dy, half * 128:(half + 1) * 128],
                                 pa3[:, 4 * t + dy:4 * t + dy + 4, 1:129],
                                 start=(dy == 0), stop=(dy == 2))
            yt = work.tile([128, 512], F32, tag="yt")
            nc.scalar.activation(out=yt[:], in_=cp[:], func=AF.Gelu)
            yo = work.tile([128, 512], F32, tag="yo")
            nc.vector.scalar_tensor_tensor(
                out=yo[:], in0=yt[:], scalar=gam_sb[:],
                in1=xh[:, 512 * t:512 * t + 512].bitcast(F32),
                op0=OP.mult, op1=OP.add)
            nc.sync.dma_start(out=y_f[half * 128:(half + 1) * 128,
                                      512 * t:512 * t + 512], in_=yo[:])


def build_nc(loop_k=None, use_cc=True, trace_sim=False, parts=None):
    nc = bacc.Bacc("TRN2", target_bir_lowering=False, debug=False,
                   num_devices=N_CORES)
    aps = {
        "xe": nc.dram_tensor("xe", [C, HE, W], F32, kind="ExternalInput").ap(),
        "wqkT": nc.dram_tensor("wqkT", [2, 128, 64], F32, kind="ExternalInput").ap(),
        "wvT": nc.dram_tensor("wvT", [2, 128, 32], F32, kind="ExternalInput").ap(),
        "bqk": nc.dram_tensor("bqk", [64], F32, kind="ExternalInput").ap(),
        "bv": nc.dram_tensor("bv", [D], F32, kind="ExternalInput").ap(),
        "wpp": nc.dram_tensor("wpp", [3, 96, C], BF16, kind="ExternalInput").ap(),
        "gamma": nc.dram_tensor("gamma", [1], F32, kind="ExternalInput").ap(),
        "y": nc.dram_tensor("y", [C, 64, W], F32, kind="ExternalOutput").ap(),
    }
    with tile.TileContext(nc, trace_sim=trace_sim) as tc:
        with ExitStack() as _ctx:
            pools = make_pools(tc, _ctx)
            if loop_k is None:
                build_body(tc, aps, pools, use_cc, parts)
            else:
                with tc.For_i(0, loop_k, 1):
                    build_body(tc, aps, pools, use_cc, parts)
    nc.finalize()
    return nc


class SpmdRunner:
    def __init__(self, nc, n_cores):
        install_neuronx_cc_hook()
        self.nc = nc
        self.n_cores = n_cores
        partition_name = nc.partition_id_tensor.name if nc.partition_id_tensor else None
        in_names, out_names, out_avals, zero_outs = [], [], [], []
        for alloc in nc.m.functions[0].allocations:
            if not isinstance(alloc, mybir.MemoryLocationSet):
                continue
            name = alloc.memorylocations[0].name
            if alloc.kind == "ExternalInput":
                if name != partition_name:
                    in_names.append(name)
            elif alloc.kind == "ExternalOutput":
                shape = tuple(alloc.tensor_shape)
                dtype = mybir.dt.np(alloc.dtype)
                out_names.append(name)
                out_avals.append(jax.core.ShapedArray(shape, dtype))
                zero_outs.append(np.zeros(shape, dtype))
        self.in_names, self.out_names = in_names, out_names
        self.out_avals, self.zero_outs = out_avals, zero_outs
        self.n_params = len(in_names)
        all_in = list(in_names) + list(out_names)
        if partition_name is not None:
            all_in.append(partition_name)

        def _body(*args):
            operands = list(args)
            if partition_name is not None:
                operands.append(partition_id_tensor())
            return tuple(_bass_exec_p.bind(
                *operands, out_avals=tuple(out_avals), in_names=tuple(all_in),
                out_names=tuple(out_names), lowering_input_output_aliases=(),
                sim_require_finite=False, sim_require_nnan=False, nc=nc))

        devices = jax.devices()[:n_cores]
        self.mesh = Mesh(np.asarray(devices), ("core",))
        n_outs = len(out_avals)
        in_specs = (PartitionSpec("core"),) * (self.n_params + n_outs)
        out_specs = (PartitionSpec("core"),) * n_outs
        self.sharded = jax.jit(
            shard_map(_body, mesh=self.mesh, in_specs=in_specs,
                      out_specs=out_specs, check_rep=False),
            keep_unused=True)

    def prepare(self, in_maps):
        n = self.n_cores
        concat_in = [
            np.concatenate([np.asarray(in_maps[c][k]) for c in range(n)], axis=0)
            for k in self.in_names
        ]
        concat_zero = [np.zeros((n * z.shape[0], *z.shape[1:]), z.dtype)
                       for z in self.zero_outs]
        sh = NamedSharding(self.mesh, PartitionSpec("core"))
        return [jax.device_put(a, sh) for a in concat_in + concat_zero]

    def run(self, args):
        outs = self.sharded(*args)
        jax.block_until_ready(outs)
        return outs

    def results(self, outs):
        n = self.n_cores
        return [
            {name: np.asarray(outs[i]).reshape(n, *self.out_avals[i].shape)[c]
             for i, name in enumerate(self.out_names)}
            for c in range(n)
        ]


_RUNNER_CACHE = {}


def get_runner(loop_k=None, use_cc=True, parts=None):
    key = (loop_k, use_cc, tuple(sorted(parts)) if parts else None)
    if key not in _RUNNER_CACHE:
        _RUNNER_CACHE[key] = SpmdRunner(build_nc(loop_k, use_cc, parts=parts),
                                        N_CORES)
    return _RUNNER_CACHE[key]


def make_in_maps(x, wq, bq, wk, bk, wv, bv, wp, gamma):
    """Shard FULL inputs into 8 per-core input dicts (with flip trick)."""
    B = x.shape[0]
    wqkT = np.ascontiguousarray(
        np.concatenate([wq.T, wk.T], axis=1).reshape(2, 128, 64), np.float32)
    wvT = np.ascontiguousarray(wv.T.reshape(2, 128, 32), np.float32)
    bqk = np.concatenate([bq, bk]).astype(np.float32)
    wpp_n = np.ascontiguousarray(
        np.transpose(wp, (2, 3, 1, 0)).reshape(3, 96, 256)).astype(ml_dtypes.bfloat16)
    wp_fl = wp[:, :, ::-1, :]
    wpp_f = np.ascontiguousarray(
        np.transpose(wp_fl, (2, 3, 1, 0)).reshape(3, 96, 256)).astype(ml_dtypes.bfloat16)
    gam = gamma.astype(np.float32)
    bvf = bv.astype(np.float32)

    in_maps = []
    for b in range(B):
        top = np.ascontiguousarray(x[b, :, 0:HE, :], np.float32)
        bot = np.ascontiguousarray(x[b, :, H - 1:H - 1 - HE:-1, :], np.float32)
        for xec, wppc in ((top, wpp_n), (bot, wpp_f)):
            in_maps.append(dict(xe=xec, wqkT=wqkT, wvT=wvT, bqk=bqk, bv=bvf,
                                wpp=wppc, gamma=gam))
    return in_maps


def assemble(results):
    """Gather per-core [256, 64, 128] outputs into [4, 256, 128, 128]."""
    B = len(results) // 2
    y = np.empty((B, C, H, W), np.float32)
    for b in range(B):
        y[b, :, 0:64, :] = results[2 * b]["y"]
        y[b, :, 64:128, :] = results[2 * b + 1]["y"][:, ::-1, :]
    return y


def kernel(**inputs):
    r = get_runner(None)
    in_maps = make_in_maps(**inputs)
    args = r.prepare(in_maps)
    outs = r.run(args)
    return assemble(r.results(outs))
